# revision 2
# baseline (speedup 1.0000x reference)
"""Trainium2 Bass kernel for the thin-plate-spline RBF layer.

reference:  out[b,n,d] = sum_m phi(|x_bn - c_bm|) * w[b,m,d],
            phi(r) = r^2 * log(r + 1e-6)

Device algorithm (per core, N sharded 8 ways):
  dist2[m,n] = sum_k a_k[m] * b_k[n]   -- rank-15 bf16 split-precision
      expansion of |x-c|^2 (coordinates centered, split into bf16 hi/lo;
      bf16 products are exact under fp32 PSUM accumulation).
      One matmul per 512-col block (nt, h, b); the four batches map to
      the four 32-row PE strips (tile_position) and run concurrently.
  L[m,n] = ln(dist2 + 5e-5)            (ScalarE, fp16 out)
  The elementwise dist2*L multiply is eliminated algebraically:
    out[b,n,d] = sum_k b_k[n] * S[(k,b,d), n],
    S = sum_m (0.5 * a_k[m] * w[m,d]) * L[m,n]   (TensorE fp16,
        batch-stacked block-diagonal weights, 60 columns).

Pipeline: blocks are ground into [128,1536] PSUM tiles (3 banks), Ln'd
as one ScalarE instruction each (fewer instructions amortize the
~352-cycle ACT overhead), and consumed by per-n-tile S chains as soon
as each L block lands.  ScalarE is the bottleneck engine (~34 us of Ln
at 1 elem/cycle/lane); everything else hides behind it.  A junk-matmul
burst at t=0 warms the PE HAM clock gate while input DMAs land.
"""
import sys

sys.path.insert(0, "/opt/trn_rl_repo")

import numpy as np
import ml_dtypes

BF16 = np.dtype(ml_dtypes.bfloat16)
FP16 = np.float16

B, M, N, NCORES = 4, 256, 32768, 8
NS = N // NCORES          # 4096 dense points per core
NT = 512                  # n-tile (one PSUM bank of fp32)
NTILES = NS // NT         # 8
HALVES = M // 128         # 2
NBLK = B * HALVES         # 8 contraction blocks of 128 per n-tile
KD = 15                   # dist2 split-precision rank
J = 5 * B * 3             # 60 stacked S columns, j = k*12 + b*3 + d
JP = 64                   # padded stationary width
DELTA = 5e-5
TW = 3                    # 512-col blocks per d2/ACT tile
NWARM = 9                 # junk warmup matmuls for the HAM clock gate

_compiled = None


def _build_nc():
    import concourse.bacc as bacc
    import concourse.mybir as mybir
    from concourse.tile import TileContext

    f32 = mybir.dt.float32
    bf = mybir.dt.bfloat16
    f16 = mybir.dt.float16
    nc = bacc.Bacc("TRN2")

    daug_d = nc.dram_tensor("daug", [128, NS], bf, kind="ExternalInput")
    cpa_d = nc.dram_tensor("cpa", [128, HALVES * 128], bf, kind="ExternalInput")
    wps_d = nc.dram_tensor("wps", [128, NBLK * JP], f16, kind="ExternalInput")
    rmat_d = nc.dram_tensor("rmat", [J, 12], f16, kind="ExternalInput")
    bcs_d = nc.dram_tensor("bcs", [J, NS], f32, kind="ExternalInput")
    out_d = nc.dram_tensor("outb", [12, NS], f32, kind="ExternalOutput")

    # block stream: (nt, h, b), b fastest => 4-way PE row-strip concurrency
    blocks = [(nt_, h, b)
              for nt_ in range(NTILES) for h in range(HALVES) for b in range(B)]
    ntiles_act = (len(blocks) + TW - 1) // TW     # 22

    with TileContext(nc) as tc:
        with (
            tc.tile_pool(name="singles", bufs=1) as singles,
            tc.tile_pool(name="lpool", bufs=6) as lpool,
            tc.tile_pool(name="zpool", bufs=2) as zpool,
            tc.tile_pool(name="d2pool", bufs=2, space="PSUM") as d2pool,
            tc.tile_pool(name="spool", bufs=1, space="PSUM") as spool,
            tc.tile_pool(name="opool", bufs=1, space="PSUM") as opool,
        ):
            delta_t = singles.tile([128, 1], f32)
            nc.vector.memset(delta_t, DELTA)
            scratch = singles.tile([128, NT], bf)
            nc.vector.memset(scratch[:], 0.0)

            # inputs, most-urgent first; two DGE queues (sync / gpsimd)
            cpa_t = singles.tile([128, HALVES * 128], bf)
            nc.sync.dma_start(out=cpa_t[:], in_=cpa_d[:])
            daug_t = singles.tile([128, NS], bf)
            for c in range(NTILES):
                csl = slice(c * NT, (c + 1) * NT)
                nc.sync.dma_start(out=daug_t[:, csl], in_=daug_d[:, csl])
            wps_t = singles.tile([128, NBLK * JP], f16)
            nc.gpsimd.dma_start(out=wps_t[:], in_=wps_d[:])
            rmat_t = singles.tile([J, 12], f16)
            nc.gpsimd.dma_start(out=rmat_t[:], in_=rmat_d[:])
            bcs_t = singles.tile([J, NS], f32)
            for c in range(NTILES):
                csl = slice(c * NT, (c + 1) * NT)
                nc.gpsimd.dma_start(out=bcs_t[:, csl], in_=bcs_d[:, csl])
            out_sb = singles.tile([12, NS], f32)

            # HAM warmup: dense junk matmul burst while input DMAs land
            wtile = d2pool.tile([128, TW * NT], f32, tag="d2")
            for _ in range(NWARM):
                nc.tensor.matmul(
                    wtile[:, :NT], scratch[:, :128], scratch[:],
                    start=True, stop=True,
                )

            s_c = None
            for t in range(ntiles_act):
                blks = blocks[t * TW:(t + 1) * TW]
                d2 = d2pool.tile([128, TW * NT], f32, tag="d2")
                for j, (nt_, h, b) in enumerate(blks):
                    nsl = slice(nt_ * NT, (nt_ + 1) * NT)
                    nc.tensor.matmul(
                        d2[:, j * NT:(j + 1) * NT],
                        cpa_t[32 * b: 32 * b + KD, h * 128:(h + 1) * 128],
                        daug_t[32 * b: 32 * b + KD, nsl],
                        start=True,
                        stop=True,
                        tile_position=(32 * b, 0),
                    )
                w = len(blks) * NT
                lt = lpool.tile([128, TW * NT], f16, tag="L")
                nc.scalar.activation(
                    out=lt[:, :w],
                    in_=d2[:, :w],
                    func=mybir.ActivationFunctionType.Ln,
                    bias=delta_t[:],
                    scale=1.0,
                )
                for j, (nt_, h, b) in enumerate(blks):
                    l = 4 * h + b
                    if l == 0:
                        s_c = spool.tile([JP, NT], f32, tag="S")
                    nc.tensor.matmul(
                        s_c[:],
                        wps_t[:, l * JP:(l + 1) * JP],
                        lt[:, j * NT:(j + 1) * NT],
                        start=(l == 0),
                        stop=(l == NBLK - 1),
                    )
                    if l == NBLK - 1:
                        nsl = slice(nt_ * NT, (nt_ + 1) * NT)
                        z_t = zpool.tile([J, NT], f16, tag="z")
                        nc.vector.tensor_mul(z_t[:], s_c[0:J, :], bcs_t[:, nsl])
                        o2 = opool.tile([12, NT], f32, tag="o2")
                        nc.tensor.matmul(o2[:], rmat_t[:], z_t[:],
                                         start=True, stop=True)
                        nc.vector.tensor_copy(out_sb[:, nsl], o2[:])
                        if nt_ % 2 == 1:
                            osl = slice((nt_ - 1) * NT, (nt_ + 1) * NT)
                            nc.sync.dma_start(out=out_d[:, osl],
                                              in_=out_sb[:, osl])

    nc.compile()
    return nc


def _split3(v):
    """3-way bf16 split of float64 array."""
    hi = v.astype(BF16)
    r1 = v - hi.astype(np.float64)
    mid = r1.astype(BF16)
    r2 = r1 - mid.astype(np.float64)
    lo = r2.astype(BF16)
    return hi, mid, lo


def _host_prep(sparse_disp, original_cp, original_dense):
    """Build per-core input maps for the device kernel."""
    x = original_dense.astype(np.float64) - 0.5   # (B, N, 3) centered
    c = original_cp.astype(np.float64) - 0.5      # (B, M, 3)
    w = sparse_disp.astype(np.float32)            # (B, M, 3)

    # ---- control-point side (shared by all cores) ----
    p = c.astype(BF16)
    q = (c - p.astype(np.float64)).astype(BF16)
    t_hi, t_mid, t_lo = _split3((c * c).sum(-1))
    ones_m = np.ones((B, M), BF16)

    # per-batch KD rows: [p x3, p x3, q x3, t_hi, t_mid, t_lo, 1, 1, 1]
    cpa_full = np.empty((B, KD, M), BF16)
    for d in range(3):
        cpa_full[:, d, :] = p[:, :, d]
        cpa_full[:, 3 + d, :] = p[:, :, d]
        cpa_full[:, 6 + d, :] = q[:, :, d]
    cpa_full[:, 9, :] = t_hi
    cpa_full[:, 10, :] = t_mid
    cpa_full[:, 11, :] = t_lo
    cpa_full[:, 12, :] = ones_m
    cpa_full[:, 13, :] = ones_m
    cpa_full[:, 14, :] = ones_m

    # stacked stationary: rows 32b..32b+KD, cols h*128..
    cpa = np.zeros((128, HALVES * 128), BF16)
    for b in range(B):
        for h in range(HALVES):
            cpa[32 * b: 32 * b + KD, h * 128:(h + 1) * 128] = \
                cpa_full[b, :, h * 128:(h + 1) * 128]

    # S-chain stationaries, fp16, l = 4h + b, packed side by side
    wps = np.zeros((128, NBLK * JP), FP16)
    c32 = c.astype(np.float32)
    a5 = np.stack(
        [c32[:, :, 0], c32[:, :, 1], c32[:, :, 2],
         (c32 * c32).sum(-1), np.ones((B, M), np.float32)],
        axis=1,
    )  # (B, 5, M)
    for h in range(HALVES):
        for b in range(B):
            l = 4 * h + b
            msl = slice(h * 128, (h + 1) * 128)
            for k in range(5):
                for d in range(3):
                    j = k * 12 + b * 3 + d
                    wps[:, l * JP + j] = 0.5 * a5[b, k, msl] * w[b, msl, d]

    rmat = np.zeros((J, 12), FP16)
    for j in range(J):
        rmat[j, j % 12] = 1.0

    # ---- dense-point side (per core) ----
    u_all = x.astype(BF16)
    v_all = (x - u_all.astype(np.float64)).astype(BF16)
    s_all = (x * x).sum(-1)

    in_maps = []
    for core in range(NCORES):
        csl = slice(core * NS, (core + 1) * NS)
        u = u_all[:, csl, :].astype(np.float32)
        v = v_all[:, csl, :].astype(np.float32)
        s_hi, s_mid, s_lo = _split3(s_all[:, csl])
        ones_n = np.ones((B, NS), BF16)

        daug_b = np.empty((B, KD, NS), BF16)
        for d in range(3):
            daug_b[:, d, :] = (-2.0 * u[:, :, d]).astype(BF16)
            daug_b[:, 3 + d, :] = (-2.0 * v[:, :, d]).astype(BF16)
            daug_b[:, 6 + d, :] = (-2.0 * u[:, :, d]).astype(BF16)
        daug_b[:, 9, :] = ones_n
        daug_b[:, 10, :] = ones_n
        daug_b[:, 11, :] = ones_n
        daug_b[:, 12, :] = s_hi
        daug_b[:, 13, :] = s_mid
        daug_b[:, 14, :] = s_lo

        daug = np.zeros((128, NS), BF16)
        for b in range(B):
            daug[32 * b: 32 * b + KD] = daug_b[b]

        xs = x[:, csl, :].astype(np.float32)
        baug5 = np.stack(
            [-2.0 * xs[:, :, 0], -2.0 * xs[:, :, 1], -2.0 * xs[:, :, 2],
             np.ones((B, NS), np.float32), (xs * xs).sum(-1)],
            axis=1,
        )  # (B, 5, NS)
        bc = np.empty((J, NS), np.float32)
        for k in range(5):
            for b in range(B):
                for d in range(3):
                    bc[k * 12 + b * 3 + d] = baug5[b, k]

        in_maps.append(
            {
                "daug": daug,
                "bcs": bc,
                "cpa": cpa,
                "wps": wps,
                "rmat": rmat,
            }
        )
    return in_maps


def _assemble(results):
    out = np.empty((B, N, 3), np.float32)
    for core, r in enumerate(results):
        o = r["outb"]  # (12, NS) rows b*3+d
        out[:, core * NS:(core + 1) * NS, :] = (
            o.reshape(B, 3, NS).transpose(0, 2, 1)
        )
    return out


def kernel(sparse_disp, original_cp, original_dense):
    global _compiled
    from concourse.bass_utils import run_bass_kernel_spmd

    if _compiled is None:
        _compiled = _build_nc()
    in_maps = _host_prep(sparse_disp, original_cp, original_dense)
    res = run_bass_kernel_spmd(_compiled, in_maps, core_ids=list(range(NCORES)))
    return _assemble(res.results)
